# revision 15
# baseline (speedup 1.0000x reference)
"""Trainium2 Bass kernel for nn_Attention_50654844289068.

Strategy (8 NeuronCores, data-parallel over batch B=8 -> 1 batch element per core):

  reference math per batch b:
    q = query @ Wq.T + bq            (S, 64)
    k = key   @ Wk.T + bk            (S, 64)
    v = value @ Wv.T + bv            (S, 64)
    s = (q @ k.T) * scale            (S, S)
    s = where(s == 0, eps, s); s = where(mask == 0, eps, s)
    w = softmax(s, axis=-1)          (S, S)   <- output 2
    att = w @ v                      (S, 64)  <- output 1

  Device-side layout choices (per core):
    - All big tensors are handled in TRANSPOSED score layout  sT[sk, sq]
      so that softmax's reduction axis (sk) lands on the partition axis,
      where the TensorEngine can reduce it for free via an appended
      ones-column in the attention@V matmul, and the e^T tiles are directly
      usable as the stationary operand of that matmul (no on-chip 2048x2048
      transpose needed).
    - The host pre-transposes query/key/value ( -> [512, S]) and the mask
      ( -> maskT[sk, sq]) while sharding, and post-transposes the weight
      output (device writes w^T).  Host-side work is only layout/dtype prep.
    - masked_fill(s==0, eps) + masked_fill(mask==0, eps):  eps = 1e-6, and
      exp(1e-6) == 1 + 1e-6.  We instead compute e = exp(scale*s*mask) so
      masked lanes give exp(0) = 1 — a 1e-6 relative difference, far below
      tolerance.  softmax has no max-subtraction: scores*scale are O(+-2),
      exp is perfectly stable there (matches jax softmax mathematically).
    - bf16 compute on PE (fp32 matmul is 4x slower), fp32 PSUM accumulate.

  Per-core phases:
    P0: project q^T,k^T [64,S] (bf16) and v [S,64] (+ ones col) from
        host-transposed inputs.
    P1: for each of 16 sk-blocks: scoresT = k_blk^T . q  (PE) ->
        s' = (scores*scale)*mask (DVE, reads PSUM) -> eT = exp(s') (ACT)
        -> accumulate attT[65, S] += [v|1]^T . eT (PE).
    P2: rowsums = attT row 64 -> 1/r (DVE) -> broadcast over partitions via
        a tiny DRAM bounce -> w^T = eT * rinv (DVE) -> DMA out;
        att = (attT rows 0:64 * rinv) transposed back on PE -> DMA out.
"""

import os
import sys
from contextlib import ExitStack

sys.path.insert(0, "/opt/trn_rl_repo")

import numpy as np
import ml_dtypes

import concourse.bacc as bacc
import concourse.bass as bass
import concourse.tile as tile
from concourse import masks, mybir
from concourse.bass_utils import run_bass_kernel_spmd

B, S, DM, DK = 8, 2048, 512, 64
NCORES = 8
P = 128
NKB = S // P          # 16 sk blocks
NCH = S // 512        # 4 sq chunks of 512
SCALE = float(DK) ** -0.5

F32 = mybir.dt.float32
BF16 = mybir.dt.bfloat16
U8 = mybir.dt.uint8
NPBF16 = ml_dtypes.bfloat16

AF = mybir.ActivationFunctionType
OP = mybir.AluOpType

LAST_RESULTS = None


def build_graph():
    nc = bacc.Bacc(
        "TRN2",
        target_bir_lowering=False,
        debug=False,
        num_devices=NCORES,
    )

    qT = nc.declare_dram_parameter("qT", [DM, S], BF16, isOutput=False)
    kT = nc.declare_dram_parameter("kT", [DM, S], BF16, isOutput=False)
    vT = nc.declare_dram_parameter("vT", [DM, S], BF16, isOutput=False)
    maskT = nc.declare_dram_parameter("maskT", [S, S], U8, isOutput=False)
    wTs = {
        t: nc.declare_dram_parameter(f"w{t}T", [DM, DK], BF16, isOutput=False)
        for t in "qkv"
    }
    biases = {
        t: nc.declare_dram_parameter(f"b{t}", [P, 1], F32, isOutput=False)
        for t in "qkv"
    }
    w_t = nc.declare_dram_parameter("w_t", [S, S], BF16, isOutput=True)
    att_t = nc.declare_dram_parameter("att_t", [DK, S], F32, isOutput=True)

    r_scr = nc.dram_tensor("r_scr", [S], F32)
    rinv_scr = nc.dram_tensor("rinv_scr", [S], BF16)

    xTs = {"q": qT, "k": kT, "v": vT}

    with tile.TileContext(nc) as tc, ExitStack() as ctx:
        persist = ctx.enter_context(tc.tile_pool(name="persist", bufs=1))
        et_pool = ctx.enter_context(tc.tile_pool(name="et", bufs=2 * NKB))

        ident_bf16 = persist.tile([P, P], BF16, tag="ident_bf16")
        masks.make_identity(nc, ident_bf16[:, :])

        # q^T and k^T duplicated onto both partition halves [0:64) and
        # [64:128) so consecutive sk blocks can run as concurrent row-group
        # matmul tiles on the PE array (K=64 uses only half the rows).
        qT_sb = persist.tile([P, S], BF16, tag="qT_sb")
        kT_sb = persist.tile([P, S], BF16, tag="kT_sb")
        v1_tiles = [
            persist.tile([P, DK + 1], BF16, tag=f"v1_{i}", name=f"v1_{i}")
            for i in range(NKB)
        ]

        # ---------------- P0: projections ----------------
        with tc.tile_pool(name="pro_in", bufs=4) as pro_in, \
             tc.tile_pool(name="pro_w", bufs=1) as pro_w, \
             tc.tile_pool(name="pro_tmp", bufs=1) as pro_tmp, \
             tc.tile_pool(name="pro_ps", bufs=4, space="PSUM") as pro_ps, \
             tc.tile_pool(name="tr_ps", bufs=2, space="PSUM") as tr_ps:

            bias_sb = {}
            for t in "qkv":
                bt = pro_w.tile([P, 1], F32, tag=f"bias_{t}")
                nc.sync.dma_start(bt[:, :], biases[t][:, :])
                bias_sb[t] = bt

            vT_sb = pro_tmp.tile([DK, S], BF16, tag="vT_sb")
            dst = {"q": qT_sb, "k": kT_sb, "v": vT_sb}

            for t in "qkv":
                dup = t in "qk"   # duplicate onto partitions [64:128)
                w_tiles = []
                for m in range(4):
                    wt = pro_w.tile([P, DK], BF16, tag=f"w_{t}_{m}")
                    nc.sync.dma_start(wt[:, :], wTs[t][m * P:(m + 1) * P, :])
                    w_tiles.append(wt)
                pss = [pro_ps.tile([P, 512], F32, tag="proj_ps", name=f"ps_{t}_{c}")
                       for c in range(NCH)]
                for m in range(4):
                    xt = pro_in.tile([P, S], BF16, tag="xin")
                    nc.sync.dma_start(xt[:, :], xTs[t][m * P:(m + 1) * P, :])
                    for c in range(NCH):
                        nc.tensor.matmul(
                            pss[c][0:DK, :],
                            w_tiles[m][:, :],
                            xt[:, c * 512:(c + 1) * 512],
                            start=(m == 0),
                            stop=(m == 3),
                            tile_position=(0, 0),
                            skip_group_check=True,
                        )
                        if dup:
                            nc.tensor.matmul(
                                pss[c][DK:2 * DK, :],
                                w_tiles[m][:, :],
                                xt[:, c * 512:(c + 1) * 512],
                                start=(m == 0),
                                stop=(m == 3),
                                tile_position=(0, DK),
                                skip_group_check=True,
                            )
                for c in range(NCH):
                    if dup:
                        nc.scalar.activation(
                            dst[t][:, c * 512:(c + 1) * 512],
                            pss[c][:, :],
                            AF.Identity,
                            bias=bias_sb[t][:, :],
                            scale=1.0,
                        )
                    else:
                        nc.scalar.activation(
                            dst[t][:, c * 512:(c + 1) * 512],
                            pss[c][0:DK, :],
                            AF.Identity,
                            bias=bias_sb[t][0:DK, :],
                            scale=1.0,
                        )

            # v natural [sk, 64] tiles with an appended ones column
            for i in range(NKB):
                pst = tr_ps.tile([P, DK], BF16, tag="tr_ps")
                nc.tensor.transpose(
                    pst[:, :], vT_sb[:, i * P:(i + 1) * P], ident_bf16[:DK, :DK]
                )
                nc.scalar.copy(v1_tiles[i][:, 0:DK], pst[:, :])
                nc.vector.memset(v1_tiles[i][:, DK:DK + 1], 1.0)


        # -------- main: four sq quarters, software-pipelined --------
        # Quarter granularity keeps the final normalize+writeout tail short:
        # quarter q's epilogue (rowsum reciprocal broadcast, w = e * rinv,
        # DMA out) overlaps quarter q+1's score/exp/AV compute.
        Q = S // 4           # 512 sq columns per quarter
        with tc.tile_pool(name="att_ps", bufs=2, space="PSUM") as att_ps, \
             tc.tile_pool(name="mask_p", bufs=20) as mask_p, \
             tc.tile_pool(name="sp_p", bufs=4) as sp_p, \
             tc.tile_pool(name="sc_ps", bufs=4, space="PSUM") as sc_ps, \
             tc.tile_pool(name="ph2", bufs=2) as ph2, \
             tc.tile_pool(name="w_p", bufs=6) as w_p:

            mask_tiles = {}
            for qt in range(4):
                c0 = qt * Q
                if qt % 2 == 0:
                    # masks are loaded per half ([128, 1024]) to keep DMA
                    # transfers chunky; each serves two quarters.
                    for kb in range(NKB):
                        mt = mask_p.tile([P, 2 * Q], U8, tag="mask",
                                         name=f"mask_{qt // 2}_{kb}")
                        nc.sync.dma_start(
                            mt[:, :],
                            maskT[kb * P:(kb + 1) * P, c0:c0 + 2 * Q],
                        )
                        mask_tiles[kb] = mt
                mq = (qt % 2) * Q

                att_acc = att_ps.tile([DK + 1, Q], F32, tag="att_acc",
                                      name=f"att_acc{qt}")
                e_tiles = []
                ps_pair = {}
                for kb in range(NKB):
                    # consecutive kb alternate PE row groups (K=64): paired
                    # score matmuls stream concurrently on the PE array.
                    rg = DK * (kb % 2)
                    ps = sc_ps.tile([P, Q], F32, tag="sc_ps",
                                    name=f"ps_{qt}_{kb}")
                    nc.tensor.matmul(
                        ps[:, :],
                        kT_sb[rg:rg + DK, kb * P:(kb + 1) * P],
                        qT_sb[rg:rg + DK, c0:c0 + Q],
                        tile_position=(rg, 0),
                    )
                    ps_pair[kb] = ps
                    if kb % 2 == 0:
                        continue
                    for kbb in (kb - 1, kb):
                        # s' = (scores * scale) * mask -> bf16
                        sp = sp_p.tile([P, Q], BF16, tag="sp")
                        nc.vector.scalar_tensor_tensor(
                            sp[:, :], ps_pair[kbb][:, :], SCALE,
                            mask_tiles[kbb][:, mq:mq + Q],
                            op0=OP.mult, op1=OP.mult,
                        )
                        e = et_pool.tile([P, Q], BF16, tag="et",
                                         name=f"e_{qt}_{kbb}")
                        nc.scalar.activation(e[:, :], sp[:, :], AF.Exp,
                                             bias=0.0, scale=1.0)
                        e_tiles.append(e)
                        nc.tensor.matmul(
                            att_acc[:, :],
                            v1_tiles[kbb][:, :],
                            e[:, :],
                            start=(kbb == 0),
                            stop=(kbb == NKB - 1),
                        )

                # ---- per-quarter epilogue ----
                att_sb = ph2.tile([DK + 1, Q], F32, tag="att_sb")
                nc.scalar.copy(att_sb[:, :], att_acc[:, :])

                # rowsums -> 1/r -> broadcast across partitions (DRAM bounce)
                nc.sync.dma_start(r_scr[c0:c0 + Q], att_sb[DK:DK + 1, :])
                rr_t = ph2.tile([P, Q // P], F32, tag="rr_t")
                nc.sync.dma_start(
                    rr_t[:, :], r_scr[c0:c0 + Q].rearrange("(p f) -> p f", p=P)
                )
                rr_inv = ph2.tile([P, Q // P], BF16, tag="rr_inv")
                with nc.allow_low_precision(reason="bf16 1/rowsum within tol"):
                    nc.vector.reciprocal(rr_inv[:, :], rr_t[:, :])
                nc.sync.dma_start(
                    rinv_scr[c0:c0 + Q].rearrange("(p f) -> p f", p=P),
                    rr_inv[:, :],
                )
                rinv_bc = ph2.tile([P, Q], BF16, tag="rinv_bc")
                nc.sync.dma_start(
                    rinv_bc[:, :],
                    rinv_scr[c0:c0 + Q]
                    .rearrange("(a s) -> a s", a=1)
                    .to_broadcast((P, Q)),
                )

                # attention weights out: w^T = e * rinv, per sk block
                for kb in range(NKB):
                    wsb = w_p.tile([P, Q], BF16, tag="wsb")
                    nc.vector.tensor_tensor(
                        wsb[:, :], e_tiles[kb][:, :], rinv_bc[:, :],
                        op=OP.mult,
                    )
                    nc.sync.dma_start(
                        w_t[kb * P:(kb + 1) * P, c0:c0 + Q], wsb[:, :]
                    )

                # attention out, transposed ([64, sq]); host transposes back
                attn_sb = ph2.tile([DK, Q], F32, tag="attn_sb")
                nc.vector.tensor_tensor(
                    attn_sb[:, :], att_sb[0:DK, :], rinv_bc[0:DK, :],
                    op=OP.mult,
                )
                nc.sync.dma_start(att_t[:, c0:c0 + Q], attn_sb[:, :])

    nc.finalize()
    return nc


_CACHE = {}


def _get_graph():
    if "nc" not in _CACHE:
        _CACHE["nc"] = build_graph()
    return _CACHE["nc"]


def make_in_maps(query, key, value, attention_mask, Wq, bq, Wk, bk, Wv, bv):
    query = np.asarray(query)
    key = np.asarray(key)
    value = np.asarray(value)
    attention_mask = np.asarray(attention_mask)
    shared = {
        "wqT": np.ascontiguousarray(np.asarray(Wq, np.float32).T).astype(NPBF16),
        "wkT": np.ascontiguousarray(np.asarray(Wk, np.float32).T).astype(NPBF16),
        "wvT": np.ascontiguousarray(np.asarray(Wv, np.float32).T).astype(NPBF16),
        "bq": np.tile(np.asarray(bq, np.float32), 2).reshape(P, 1),
        "bk": np.tile(np.asarray(bk, np.float32), 2).reshape(P, 1),
        "bv": np.tile(np.asarray(bv, np.float32), 2).reshape(P, 1),
    }
    in_maps = []
    for b in range(B):
        in_maps.append(
            {
                "qT": np.ascontiguousarray(query[b].T).astype(NPBF16),
                "kT": np.ascontiguousarray(key[b].T).astype(NPBF16),
                "vT": np.ascontiguousarray(value[b].T).astype(NPBF16),
                "maskT": np.ascontiguousarray(attention_mask[b].T).astype(np.uint8),
                **shared,
            }
        )
    return in_maps


def kernel(query, key, value, attention_mask, Wq, bq, Wk, bk, Wv, bv):
    global LAST_RESULTS
    nc = _get_graph()
    in_maps = make_in_maps(
        query, key, value, attention_mask, Wq, bq, Wk, bk, Wv, bv
    )
    res = run_bass_kernel_spmd(nc, in_maps, core_ids=list(range(NCORES)))
    LAST_RESULTS = res
    att = np.stack(
        [
            np.asarray(res.results[c]["att_t"], np.float32).T
            for c in range(NCORES)
        ]
    )
    w = np.stack(
        [
            np.asarray(res.results[c]["w_t"]).astype(np.float32).T
            for c in range(NCORES)
        ]
    )
    return np.ascontiguousarray(att), np.ascontiguousarray(w)


# revision 16
# speedup vs baseline: 1.2518x; 1.2518x over previous
"""Trainium2 Bass kernel for nn_Attention_50654844289068.

Strategy (8 NeuronCores, data-parallel over batch B=8 -> 1 batch element per core):

  reference math per batch b:
    q = query @ Wq.T + bq            (S, 64)
    k = key   @ Wk.T + bk            (S, 64)
    v = value @ Wv.T + bv            (S, 64)
    s = (q @ k.T) * scale            (S, S)
    s = where(s == 0, eps, s); s = where(mask == 0, eps, s)
    w = softmax(s, axis=-1)          (S, S)   <- output 2
    att = w @ v                      (S, 64)  <- output 1

  Device-side layout choices (per core):
    - All big tensors are handled in TRANSPOSED score layout  sT[sk, sq]
      so that softmax's reduction axis (sk) lands on the partition axis,
      where the TensorEngine can reduce it for free via an appended
      ones-column in the attention@V matmul, and the e^T tiles are directly
      usable as the stationary operand of that matmul (no on-chip 2048x2048
      transpose needed).
    - The host pre-transposes query/key/value ( -> [512, S]) and the mask
      ( -> maskT[sk, sq]) while sharding, and post-transposes the weight
      output (device writes w^T).  Host-side work is only layout/dtype prep.
    - masked_fill(s==0, eps) + masked_fill(mask==0, eps):  eps = 1e-6, and
      exp(1e-6) == 1 + 1e-6.  We instead compute e = exp(scale*s*mask) so
      masked lanes give exp(0) = 1 — a 1e-6 relative difference, far below
      tolerance.  softmax has no max-subtraction: scores*scale are O(+-2),
      exp is perfectly stable there (matches jax softmax mathematically).
    - bf16 compute on PE (fp32 matmul is 4x slower), fp32 PSUM accumulate.

  Per-core phases:
    P0: project q^T,k^T [64,S] (bf16) and v [S,64] (+ ones col) from
        host-transposed inputs.
    P1: for each of 16 sk-blocks: scoresT = k_blk^T . q  (PE) ->
        s' = (scores*scale)*mask (DVE, reads PSUM) -> eT = exp(s') (ACT)
        -> accumulate attT[65, S] += [v|1]^T . eT (PE).
    P2: rowsums = attT row 64 -> 1/r (DVE) -> broadcast over partitions via
        a tiny DRAM bounce -> w^T = eT * rinv (DVE) -> DMA out;
        att = (attT rows 0:64 * rinv) transposed back on PE -> DMA out.
"""

import os
import sys
from contextlib import ExitStack

sys.path.insert(0, "/opt/trn_rl_repo")

import numpy as np
import ml_dtypes

import concourse.bacc as bacc
import concourse.bass as bass
import concourse.tile as tile
from concourse import masks, mybir
from concourse.bass_utils import run_bass_kernel_spmd

B, S, DM, DK = 8, 2048, 512, 64
NCORES = 8
P = 128
NKB = S // P          # 16 sk blocks
NCH = S // 512        # 4 sq chunks of 512
SCALE = float(DK) ** -0.5

F32 = mybir.dt.float32
BF16 = mybir.dt.bfloat16
U8 = mybir.dt.uint8
NPBF16 = ml_dtypes.bfloat16

AF = mybir.ActivationFunctionType
OP = mybir.AluOpType

LAST_RESULTS = None


def build_graph():
    nc = bacc.Bacc(
        "TRN2",
        target_bir_lowering=False,
        debug=False,
        num_devices=NCORES,
    )

    qT = nc.declare_dram_parameter("qT", [DM, S], BF16, isOutput=False)
    kT = nc.declare_dram_parameter("kT", [DM, S], BF16, isOutput=False)
    vT = nc.declare_dram_parameter("vT", [DM, S], BF16, isOutput=False)
    maskT = nc.declare_dram_parameter("maskT", [S, S], U8, isOutput=False)
    wTs = {
        t: nc.declare_dram_parameter(f"w{t}T", [DM, DK], BF16, isOutput=False)
        for t in "qkv"
    }
    biases = {
        t: nc.declare_dram_parameter(f"b{t}", [P, 1], F32, isOutput=False)
        for t in "qkv"
    }
    w_t = nc.declare_dram_parameter("w_t", [S, S], BF16, isOutput=True)
    att_t = nc.declare_dram_parameter("att_t", [DK, S], F32, isOutput=True)

    r_scr = nc.dram_tensor("r_scr", [S], F32)
    rinv_scr = nc.dram_tensor("rinv_scr", [S], BF16)

    xTs = {"q": qT, "k": kT, "v": vT}

    with tile.TileContext(nc) as tc, ExitStack() as ctx:
        persist = ctx.enter_context(tc.tile_pool(name="persist", bufs=1))
        et_pool = ctx.enter_context(tc.tile_pool(name="et", bufs=2 * NKB))

        ident_bf16 = persist.tile([P, P], BF16, tag="ident_bf16")
        masks.make_identity(nc, ident_bf16[:, :])

        # q^T and k^T duplicated onto both partition halves [0:64) and
        # [64:128) so consecutive sk blocks can run as concurrent row-group
        # matmul tiles on the PE array (K=64 uses only half the rows).
        qT_sb = persist.tile([P, S], BF16, tag="qT_sb")
        kT_sb = persist.tile([P, S], BF16, tag="kT_sb")
        v1_tiles = [
            persist.tile([P, DK + 1], BF16, tag=f"v1_{i}", name=f"v1_{i}")
            for i in range(NKB)
        ]

        # ---------------- P0: projections ----------------
        with tc.tile_pool(name="pro_in", bufs=4) as pro_in, \
             tc.tile_pool(name="pro_w", bufs=1) as pro_w, \
             tc.tile_pool(name="pro_tmp", bufs=1) as pro_tmp, \
             tc.tile_pool(name="pro_ps", bufs=4, space="PSUM") as pro_ps, \
             tc.tile_pool(name="tr_ps", bufs=2, space="PSUM") as tr_ps:

            bias_sb = {}
            for t in "qkv":
                bt = pro_w.tile([P, 1], F32, tag=f"bias_{t}")
                nc.sync.dma_start(bt[:, :], biases[t][:, :])
                bias_sb[t] = bt

            vT_sb = pro_tmp.tile([DK, S], BF16, tag="vT_sb")
            dst = {"q": qT_sb, "k": kT_sb, "v": vT_sb}

            for t in "qkv":
                dup = t in "qk"   # duplicate onto partitions [64:128)
                w_tiles = []
                for m in range(4):
                    wt = pro_w.tile([P, DK], BF16, tag=f"w_{t}_{m}")
                    nc.sync.dma_start(wt[:, :], wTs[t][m * P:(m + 1) * P, :])
                    w_tiles.append(wt)
                pss = [pro_ps.tile([P, 512], F32, tag="proj_ps", name=f"ps_{t}_{c}")
                       for c in range(NCH)]
                for m in range(4):
                    xt = pro_in.tile([P, S], BF16, tag="xin")
                    nc.sync.dma_start(xt[:, :], xTs[t][m * P:(m + 1) * P, :])
                    for c in range(NCH):
                        nc.tensor.matmul(
                            pss[c][0:DK, :],
                            w_tiles[m][:, :],
                            xt[:, c * 512:(c + 1) * 512],
                            start=(m == 0),
                            stop=(m == 3),
                            tile_position=(0, 0),
                            skip_group_check=True,
                        )
                        if dup:
                            nc.tensor.matmul(
                                pss[c][DK:2 * DK, :],
                                w_tiles[m][:, :],
                                xt[:, c * 512:(c + 1) * 512],
                                start=(m == 0),
                                stop=(m == 3),
                                tile_position=(0, DK),
                                skip_group_check=True,
                            )
                for c in range(NCH):
                    if dup:
                        nc.scalar.activation(
                            dst[t][:, c * 512:(c + 1) * 512],
                            pss[c][:, :],
                            AF.Identity,
                            bias=bias_sb[t][:, :],
                            scale=1.0,
                        )
                    else:
                        nc.scalar.activation(
                            dst[t][:, c * 512:(c + 1) * 512],
                            pss[c][0:DK, :],
                            AF.Identity,
                            bias=bias_sb[t][0:DK, :],
                            scale=1.0,
                        )

            # v natural [sk, 64] tiles with an appended ones column
            for i in range(NKB):
                pst = tr_ps.tile([P, DK], BF16, tag="tr_ps")
                nc.tensor.transpose(
                    pst[:, :], vT_sb[:, i * P:(i + 1) * P], ident_bf16[:DK, :DK]
                )
                nc.scalar.copy(v1_tiles[i][:, 0:DK], pst[:, :])
                nc.vector.memset(v1_tiles[i][:, DK:DK + 1], 1.0)


        # -------- main: two sq halves; kb-pairs with lookahead --------
        # Score matmuls for a kb pair are emitted adjacently (alternating PE
        # row groups -> they stream concurrently on the half-filled array)
        # and one pair AHEAD of the STT/exp/AV consumers, so the PE stream
        # never stalls on the elementwise chain.
        H = S // 2
        HCH = H // 512
        NPAIR = NKB // 2
        PL = 1  # pairs of lookahead
        with tc.tile_pool(name="att_ps", bufs=1, space="PSUM") as att_ps, \
             tc.tile_pool(name="mask_p", bufs=20) as mask_p, \
             tc.tile_pool(name="sp_p", bufs=4) as sp_p, \
             tc.tile_pool(name="sc_ps", bufs=3, space="PSUM") as sc_ps, \
             tc.tile_pool(name="ph2", bufs=2) as ph2, \
             tc.tile_pool(name="w_p", bufs=6) as w_p:

            for h in range(2):
                c0 = h * H
                att_acc = [
                    att_ps.tile([DK + 1, 512], F32, tag=f"att_acc{c}",
                                name=f"att_acc{h}_{c}")
                    for c in range(HCH)
                ]
                e_tiles = {}
                ps_tiles = {}
                mask_tiles = {}

                def emit_scores(pj, h=h, c0=c0, e_tiles=e_tiles,
                                ps_tiles=ps_tiles, mask_tiles=mask_tiles):
                    for kb in (2 * pj, 2 * pj + 1):
                        mt = mask_p.tile([P, H], U8, tag="mask",
                                         name=f"mask_{h}_{kb}")
                        nc.sync.dma_start(
                            mt[:, :], maskT[kb * P:(kb + 1) * P, c0:c0 + H]
                        )
                        mask_tiles[kb] = mt
                        ps_tiles[kb] = sc_ps.tile([P, H], F32, tag="sc_ps",
                                                  name=f"ps_{h}_{kb}")
                    for cc in range(HCH):
                        for kb in (2 * pj, 2 * pj + 1):
                            rg = DK * (kb % 2)
                            nc.tensor.matmul(
                                ps_tiles[kb][:, cc * 512:(cc + 1) * 512],
                                kT_sb[rg:rg + DK, kb * P:(kb + 1) * P],
                                qT_sb[rg:rg + DK,
                                      c0 + cc * 512:c0 + (cc + 1) * 512],
                                tile_position=(rg, 0),
                            )

                def emit_consume(pj, h=h, e_tiles=e_tiles, ps_tiles=ps_tiles,
                                 mask_tiles=mask_tiles, att_acc=att_acc):
                    for kb in (2 * pj, 2 * pj + 1):
                        sp = sp_p.tile([P, H], BF16, tag="sp")
                        nc.vector.scalar_tensor_tensor(
                            sp[:, :], ps_tiles.pop(kb)[:, :], SCALE,
                            mask_tiles.pop(kb)[:, :],
                            op0=OP.mult, op1=OP.mult,
                        )
                        e = et_pool.tile([P, H], BF16, tag="et",
                                         name=f"e_{h}_{kb}")
                        nc.scalar.activation(e[:, :], sp[:, :], AF.Exp,
                                             bias=0.0, scale=1.0)
                        e_tiles[kb] = e
                        for cc in range(HCH):
                            nc.tensor.matmul(
                                att_acc[cc][:, :],
                                v1_tiles[kb][:, :],
                                e[:, cc * 512:(cc + 1) * 512],
                                start=(kb == 0),
                                stop=(kb == NKB - 1),
                            )

                for pj in range(NPAIR + PL):
                    if pj < NPAIR:
                        emit_scores(pj)
                    if pj >= PL:
                        emit_consume(pj - PL)

                # ---- per-half epilogue ----
                att_sb = ph2.tile([DK + 1, H], F32, tag="att_sb")
                for cc in range(HCH):
                    nc.scalar.copy(att_sb[:, cc * 512:(cc + 1) * 512],
                                   att_acc[cc][:, :])

                # rowsums -> 1/r -> broadcast across partitions (DRAM bounce)
                nc.sync.dma_start(r_scr[c0:c0 + H], att_sb[DK:DK + 1, :])
                rr_t = ph2.tile([P, H // P], F32, tag="rr_t")
                nc.sync.dma_start(
                    rr_t[:, :], r_scr[c0:c0 + H].rearrange("(p f) -> p f", p=P)
                )
                rr_inv = ph2.tile([P, H // P], BF16, tag="rr_inv")
                with nc.allow_low_precision(reason="bf16 1/rowsum within tol"):
                    nc.vector.reciprocal(rr_inv[:, :], rr_t[:, :])
                nc.sync.dma_start(
                    rinv_scr[c0:c0 + H].rearrange("(p f) -> p f", p=P),
                    rr_inv[:, :],
                )
                rinv_bc = ph2.tile([P, H], BF16, tag="rinv_bc")
                nc.sync.dma_start(
                    rinv_bc[:, :],
                    rinv_scr[c0:c0 + H]
                    .rearrange("(a s) -> a s", a=1)
                    .to_broadcast((P, H)),
                )

                # attention weights out: w^T = e * rinv, per sk block
                for kb in range(NKB):
                    wsb = w_p.tile([P, H], BF16, tag="wsb")
                    nc.vector.tensor_tensor(
                        wsb[:, :], e_tiles[kb][:, :], rinv_bc[:, :],
                        op=OP.mult,
                    )
                    nc.sync.dma_start(
                        w_t[kb * P:(kb + 1) * P, c0:c0 + H], wsb[:, :]
                    )

                # attention out, transposed ([64, sq]); host transposes back
                attn_sb = ph2.tile([DK, H], F32, tag="attn_sb")
                nc.vector.tensor_tensor(
                    attn_sb[:, :], att_sb[0:DK, :], rinv_bc[0:DK, :],
                    op=OP.mult,
                )
                nc.sync.dma_start(att_t[:, c0:c0 + H], attn_sb[:, :])

    nc.finalize()
    return nc


_CACHE = {}


def _get_graph():
    if "nc" not in _CACHE:
        _CACHE["nc"] = build_graph()
    return _CACHE["nc"]


def make_in_maps(query, key, value, attention_mask, Wq, bq, Wk, bk, Wv, bv):
    query = np.asarray(query)
    key = np.asarray(key)
    value = np.asarray(value)
    attention_mask = np.asarray(attention_mask)
    shared = {
        "wqT": np.ascontiguousarray(np.asarray(Wq, np.float32).T).astype(NPBF16),
        "wkT": np.ascontiguousarray(np.asarray(Wk, np.float32).T).astype(NPBF16),
        "wvT": np.ascontiguousarray(np.asarray(Wv, np.float32).T).astype(NPBF16),
        "bq": np.tile(np.asarray(bq, np.float32), 2).reshape(P, 1),
        "bk": np.tile(np.asarray(bk, np.float32), 2).reshape(P, 1),
        "bv": np.tile(np.asarray(bv, np.float32), 2).reshape(P, 1),
    }
    in_maps = []
    for b in range(B):
        in_maps.append(
            {
                "qT": np.ascontiguousarray(query[b].T).astype(NPBF16),
                "kT": np.ascontiguousarray(key[b].T).astype(NPBF16),
                "vT": np.ascontiguousarray(value[b].T).astype(NPBF16),
                "maskT": np.ascontiguousarray(attention_mask[b].T).astype(np.uint8),
                **shared,
            }
        )
    return in_maps


def kernel(query, key, value, attention_mask, Wq, bq, Wk, bk, Wv, bv):
    global LAST_RESULTS
    nc = _get_graph()
    in_maps = make_in_maps(
        query, key, value, attention_mask, Wq, bq, Wk, bk, Wv, bv
    )
    res = run_bass_kernel_spmd(nc, in_maps, core_ids=list(range(NCORES)))
    LAST_RESULTS = res
    att = np.stack(
        [
            np.asarray(res.results[c]["att_t"], np.float32).T
            for c in range(NCORES)
        ]
    )
    w = np.stack(
        [
            np.asarray(res.results[c]["w_t"]).astype(np.float32).T
            for c in range(NCORES)
        ]
    )
    return np.ascontiguousarray(att), np.ascontiguousarray(w)
